# revision 48
# baseline (speedup 1.0000x reference)
"""MoE layer (74 experts, top-6 routing) on 8 Trainium2 NeuronCores.

Strategy (expert-parallel, host-side dispatch):
  - Host computes the router in numpy f32 (logits -> softmax -> top-6 ->
    renormalized gates) plus the load-balancing loss.
  - Experts are sharded across the 8 cores: 74 = 8 * 9.25, so each core gets
    9 full experts plus one H/4 quarter of expert 72 (cores 0-3) or 73
    (cores 4-7). Quarter outputs sum across cores to the full FFN (gelu is
    elementwise in H). Zero padding waste; every core runs the identical
    SPMD program.
  - Per expert the host gathers its routed tokens (capacity 128, zero-padded)
    into a transposed activation bank xgt and packs W1/W2 into a single
    stream-ordered blob (per m-block: w1kc0|w1kc1|w2).
  - Device, per expert slot (fp16 matmuls, f32 PSUM accumulation):
        HT[m]  = gelu(W1^T-block-m @ XgT)     2 m-blocks per PSUM tile,
                                              one gelu per pair
        Y     += HT[m]^T-as-lhsT @ W2[m]      software-pipelined one pack
                                              behind the gelu stream
        Y -> DRAM in f32                      per expert slot
    Weight DMA streams in compute order, chunked 4 m-blocks per transfer.
  - Host applies the f32 gates and scatter-adds each expert's Y rows back to
    token positions (the unshard step), then adds the gate-weighted b2 term.

Self-contained: hardcodes shapes from the problem spec (B=1, S=512, D=256,
H=2048, E=74, TOPK=6).
"""

import numpy as np

import concourse.bass as bass
import concourse.mybir as mybir
import concourse.tile as tile
from concourse import bacc
from concourse.bass_utils import run_bass_kernel_spmd

S, D, H, E, TOPK = 512, 256, 2048, 74, 6
NCORES = 8
CAP = 96          # token capacity per expert (counts ~41 +- 7, max seen 67)
KC = D // 128     # 2 contraction blocks for gemm1
MC = H // 128     # 16 m-blocks (hidden) for gemm2
TB = S // 128     # 4 token blocks for combine

# Compute dtype for matmul operands. float32 is exact but 4x slower on the PE
# and 2x the HBM traffic; bfloat16/float16 run at full rate.
COMPUTE_DT = mybir.dt.float16
_NP_DT = {mybir.dt.float32: np.float32, mybir.dt.float16: np.float16}
try:
    import ml_dtypes
    _NP_DT[mybir.dt.bfloat16] = ml_dtypes.bfloat16
except ImportError:
    pass

_NC_CACHE: dict = {}


# Slot layout: 9 full experts + 1 quarter expert per core.
# 74 = 8*9.25 exactly: experts 0..71 are full slots (9 per core); expert 72 is
# split into 4 H-quarters on cores 0-3, expert 73 on cores 4-7. A quarter slot
# runs the same math on H/4 hidden units; summing quarter outputs over cores
# reconstructs the full FFN (gelu is elementwise in H). Zero dummies.
NFULL = 9                 # full expert slots per core
EPC = NFULL + 1           # +1 quarter slot
MCQ = MC // 4             # m-blocks in the quarter slot
GRP = KC * 128 + D        # blob columns per m-block group: w1kc0|w1kc1|w2
WCOL_FULL = MC * GRP      # weight-blob columns per full slot
WCOL_Q = MCQ * GRP        # columns for the quarter slot
WCOLS = NFULL * WCOL_FULL + WCOL_Q
WCHUNK = 4                # m-block groups per DMA chunk


def _slot_mcs(e):
    return MC if e < NFULL else MCQ


def _build_nc(cap: int, dt, zero_b1: bool):
    f32 = mybir.dt.float32
    nc = bacc.Bacc("TRN2", target_bir_lowering=False, debug=False,
                   num_devices=NCORES)
    xgt_d = nc.dram_tensor("xgt", [128, EPC * KC * cap], dt,
                           kind="ExternalInput").ap()
    wts_d = nc.dram_tensor("wts", [128, WCOLS], dt, kind="ExternalInput").ap()
    if not zero_b1:
        # b1 rows for the K=1 bias matmul: [1, EPC*MC*128]
        b1_d = nc.dram_tensor("b1r", [1, EPC * MC * 128], dt,
                              kind="ExternalInput").ap()
    out_d = nc.dram_tensor("out", [EPC * cap, D], f32,
                           kind="ExternalOutput").ap()

    with tile.TileContext(nc) as tc:
        with tc.tile_pool(name="singles", bufs=1) as singles, \
             tc.tile_pool(name="wp", bufs=5) as wp, \
             tc.tile_pool(name="htp", bufs=3) as htp, \
             tc.tile_pool(name="yp", bufs=2) as yp, \
             tc.tile_pool(name="pph", bufs=3, space="PSUM") as pph, \
             tc.tile_pool(name="ppy", bufs=2, space="PSUM") as ppy:

            xgt_sb = singles.tile([128, EPC * KC * cap], dt)
            nc.sync.dma_start(out=xgt_sb, in_=xgt_d)
            if not zero_b1:
                b1_sb = singles.tile([1, EPC * MC * 128], dt)
                ones_sb = singles.tile([1, cap], dt)
                nc.vector.memset(ones_sb, 1.0)
            def combine_pieces(e, y_ps):
                """ship this expert's gated-less Y to DRAM in f32; the host
                applies gates and scatter-adds (the unshard step)"""
                y_sb = yp.tile([cap, D], f32, tag="ysb", name=f"ysb{e}")

                def ycopy():
                    nc.vector.tensor_copy(y_sb, y_ps)

                def yout():
                    nc.sync.dma_start(out=out_d[e * cap:(e + 1) * cap, :],
                                      in_=y_sb)

                return [ycopy, yout]

            work = []     # deferred closures from the previous expert
            for e in range(EPC):
                mcs = _slot_mcs(e)
                npk = mcs // 2
                base = e * WCOL_FULL if e < NFULL else NFULL * WCOL_FULL
                wt_sb = wp.tile([128, WCOL_FULL], dt, tag="wt",
                                name=f"wt{e}")
                # stream in compute order, chunked so compute starts on the
                # first chunk while the rest is still in flight
                for q0 in range(0, mcs, WCHUNK):
                    c0 = q0 * GRP
                    c1 = min(q0 + WCHUNK, mcs) * GRP
                    nc.sync.dma_start(out=wt_sb[:, c0:c1],
                                      in_=wts_d[:, base + c0:base + c1])
                if e == 0 and not zero_b1:
                    nc.sync.dma_start(out=b1_sb, in_=b1_d)

                def w1_ap(kc, mc, wt_sb=wt_sb):
                    c0 = mc * GRP + kc * 128
                    return wt_sb[:, c0:c0 + 128]

                def w2_ap(mc, wt_sb=wt_sb):
                    c0 = mc * GRP + KC * 128
                    return wt_sb[:, c0:c0 + D]

                y_ps = ppy.tile([cap, D], f32, tag="yps", name=f"yps{e}")
                hts = []
                for p in range(npk):
                    # 2 m-blocks per psum tile, one per 2KB bank (a matmul
                    # accumulation group may not share its zero-region bank)
                    h_ps = pph.tile([128, 2, 512], f32, tag="hps",
                                    name=f"hps{e}_{p}")
                    for i in range(2):
                        mc = 2 * p + i
                        for kc in range(KC):
                            nc.tensor.matmul(
                                h_ps[:, i, 0:cap],
                                lhsT=w1_ap(kc, mc),
                                rhs=xgt_sb[:, (e * KC + kc) * cap:
                                           (e * KC + kc + 1) * cap],
                                start=(kc == 0),
                                stop=(kc == KC - 1 and zero_b1))
                        if not zero_b1:
                            r0 = (e * MC + mc) * 128
                            nc.tensor.matmul(
                                h_ps[:, i, 0:cap],
                                lhsT=b1_sb[:, r0:r0 + 128],
                                rhs=ones_sb,
                                start=False, stop=True)
                    ht_sb = htp.tile([128, 2, cap], dt, tag="ht",
                                     name=f"ht{e}_{p}")
                    nc.scalar.activation(
                        ht_sb, h_ps[:, :, 0:cap],
                        mybir.ActivationFunctionType.Gelu, scale=1.0)
                    hts.append(ht_sb)
                    if work:
                        work.pop(0)()         # prev expert's deferred piece
                    if p >= 1:
                        for i in range(2):
                            mc = 2 * (p - 1) + i
                            nc.tensor.matmul(
                                y_ps, lhsT=hts[p - 1][:, i, :],
                                rhs=w2_ap(mc),
                                start=(mc == 0), stop=False)

                def tail_y(y_ps=y_ps, hts=hts, w2_ap=w2_ap, mcs=mcs, npk=npk):
                    for i in range(2):
                        mc = 2 * (npk - 1) + i
                        nc.tensor.matmul(y_ps, lhsT=hts[npk - 1][:, i, :],
                                         rhs=w2_ap(mc),
                                         start=(mc == 0),
                                         stop=(mc == mcs - 1))
                for fn in work:               # anything not yet drained
                    fn()
                work = [tail_y] + combine_pieces(e, y_ps)

            for fn in work:
                fn()

    nc.compile()
    return nc


def _route(x2: np.ndarray, Wr: np.ndarray):
    """Router in f32 numpy: softmax over experts, top-6, renormalize."""
    logits = x2 @ Wr                                     # [S, E]
    m = logits.max(axis=1, keepdims=True)
    p = np.exp(logits - m, dtype=np.float32)
    p /= p.sum(axis=1, keepdims=True)
    idx = np.argpartition(-p, TOPK - 1, axis=1)[:, :TOPK]  # [S, K] top-6 set
    sc = np.take_along_axis(p, idx, axis=1)
    sc = sc / sc.sum(axis=1, keepdims=True)
    gates = np.zeros((S, E), np.float32)
    np.put_along_axis(gates, idx, sc, axis=1)
    return gates, idx


def _prepare(x, Wr, W1, b1, W2, b2, build=True):
    x = np.asarray(x, np.float32)
    Wr = np.asarray(Wr, np.float32)
    W1 = np.asarray(W1, np.float32)
    b1 = np.asarray(b1, np.float32)
    W2 = np.asarray(W2, np.float32)
    b2 = np.asarray(b2, np.float32)
    x2 = x.reshape(S, D)

    gates, idx = _route(x2, Wr)
    counts = gates.sum(axis=0)
    load_loss = np.float32(
        np.mean((counts / counts.sum() * E - 1.0) ** 2, dtype=np.float32))

    # token list per expert
    toklist = [[] for _ in range(E)]
    for t in range(S):
        for e in idx[t]:
            toklist[e].append(t)
    maxcount = max(len(tl) for tl in toklist)
    cap = CAP if maxcount <= CAP else (maxcount + 15) // 16 * 16

    np_dt = _NP_DT[COMPUTE_DT]
    Hq = H // 4
    xgt = np.zeros((NCORES, 128, EPC, KC, cap), np.float32)
    wts = np.zeros((NCORES, 128, WCOLS), np.float32)
    b1r = np.zeros((NCORES, 1, EPC * MC * 128), np.float32)
    zero_b1 = not b1.any()

    def fill_tokens(c, j, ge):
        tk = np.asarray(toklist[ge], np.int64)
        m = len(tk)
        if m:
            xs = x2[tk].T                      # [D, m]
            xgt[c, :, j, 0, :m] = xs[:128]
            xgt[c, :, j, 1, :m] = xs[128:]

    for c in range(NCORES):
        for j in range(NFULL):
            ge = c * NFULL + j
            fill_tokens(c, j, ge)
            base = j * WCOL_FULL
            w1i = W1[ge].reshape(KC, 128, MC, 128).transpose(1, 2, 0, 3)
            w2i = W2[ge].reshape(MC, 128, D).transpose(1, 0, 2)
            blob = np.concatenate(
                [w1i.reshape(128, MC, KC * 128), w2i], axis=2)
            wts[c, :, base:base + WCOL_FULL] = blob.reshape(128, WCOL_FULL)
            b1r[c, 0, j * MC * 128:(j + 1) * MC * 128] = b1[ge]
        # quarter slot: expert 72 quartered on cores 0-3, expert 73 on 4-7
        ge, qi = (72, c) if c < 4 else (73, c - 4)
        j = NFULL
        fill_tokens(c, j, ge)
        base = NFULL * WCOL_FULL
        hsl = slice(qi * Hq, (qi + 1) * Hq)
        w1i = W1[ge, :, hsl].reshape(KC, 128, MCQ, 128).transpose(1, 2, 0, 3)
        w2i = W2[ge, hsl].reshape(MCQ, 128, D).transpose(1, 0, 2)
        blob = np.concatenate([w1i.reshape(128, MCQ, KC * 128), w2i], axis=2)
        wts[c, :, base:base + WCOL_Q] = blob.reshape(128, WCOL_Q)
        b1r[c, 0, j * MC * 128:j * MC * 128 + MCQ * 128] = b1[ge, hsl]

    nc = None
    if build:
        key = (cap, COMPUTE_DT, zero_b1)
        if key not in _NC_CACHE:
            _NC_CACHE[key] = _build_nc(cap, COMPUTE_DT, zero_b1)
        nc = _NC_CACHE[key]

    in_maps = [{
        "xgt": np.ascontiguousarray(
            xgt[c].reshape(128, EPC * KC * cap)).astype(np_dt),
        "wts": np.ascontiguousarray(wts[c]).astype(np_dt),
    } for c in range(NCORES)]
    if not zero_b1:
        for c in range(NCORES):
            in_maps[c]["b1r"] = np.ascontiguousarray(b1r[c]).astype(np_dt)
    return nc, in_maps, gates, load_loss, cap, toklist


def _slot_expert(c, j):
    return c * NFULL + j if j < NFULL else (72 if c < 4 else 73)


def kernel(x, Wr, W1, b1, W2, b2):
    b2 = np.asarray(b2, np.float32)
    nc, in_maps, gates, load_loss, cap, toklist = \
        _prepare(x, Wr, W1, b1, W2, b2)
    res = run_bass_kernel_spmd(nc, in_maps, core_ids=list(range(NCORES)))
    out = np.zeros((S, D), np.float32)
    for c in range(NCORES):
        Y = res.results[c]["out"].reshape(EPC, cap, D)
        for j in range(EPC):
            ge = _slot_expert(c, j)
            tk = toklist[ge]
            m = len(tk)
            if m:
                out[tk] += gates[tk, ge][:, None] * Y[j, :m]
    out += gates @ b2                # gate-weighted second bias (zeros in spec)
    return out.reshape(1, S, D), load_loss
